# revision 22
# baseline (speedup 1.0000x reference)
"""Trainium2 Bass kernel for the 2-hop GNN (GCN + SAGE + BatchNorm) problem.

Strategy (8 NeuronCores, SPMD single program):
  - Destination (user-node) sharding with load-balanced 62-dst windows: core k
    owns output rows [k*12500, (k+1)*12500), assigned to NW=208 windows via a
    greedy vector bin-pack so every (window, relation) holds <= B*128 edges.
  - Host prep is integer index manipulation + layout packing only (sort/bucket
    edges, bincount degrees, compose gather indices). FP math runs on device.
  - Gather table T (DRAM, bf16) = [newF rows 0..50048 | emb_table rows
    50048..100096 | zero row]; edge sources composed through no_Nidx on host.
  - Scatter-add via one-hot matmuls: S[e,d] = (colw[e]==d)*edis[e], built per
    4-window group with broadcast APs on DVE; TensorE accumulates
    gathered^T @ S into PSUM; ACT drains PSUM into bf16 agg tiles in SBUF.
  - Indirect gathers batched (3D dst AP, one descriptor per row).
  - deg/cnt from host bincount (integer metadata); dis, 1/cnt on device.
  - BN stats via AllReduce (with correction for unassigned window slots);
    output written [H, LP], transposed/unpermuted on host.
"""

import numpy as np
import ml_dtypes

import concourse.bass as bass
import concourse.bacc as bacc
import concourse.tile as tile
import concourse.mybir as mybir
from concourse import bass_utils
from concourse.bass import IndirectOffsetOnAxis

F32 = mybir.dt.float32
BF16 = mybir.dt.bfloat16
I32 = mybir.dt.int32
NPBF16 = ml_dtypes.bfloat16

# Problem constants
U1 = 50000
U2 = 50000
U = 100000
C = 200000
E = 1000000
ED = 85
DC = 64
H = 128
NH = 2
EMB_BLOCKS = [(0, 0), (3, 18), (7, 37), (8, 53), (9, 69)]
PASS_COLS = [(1, 16), (2, 17), (4, 34), (5, 35), (6, 36)]

NCORES = 8
L = 12500
WIN = 62             # destinations per window
NW = 208             # windows per core
LP = NW * WIN        # 12896 agg columns per core
NEMPTY = LP - L      # 396 unassigned slots per core
NFT = 391            # newF row-tiles
NFROWS = NFT * 128   # 50048
EMB0 = NFROWS        # T row offset of emb region
TR = EMB0 + NFROWS   # 100096 real T rows
ZROW = TR            # zero-row index
TROWS = TR + 8       # padded T rows
NLOCG = (LP + 127) // 128       # 101 local gather subtiles
NLOCP = ((NLOCG + 3) // 4) * 4  # 104 (pad for gather batch of 4)
NT = (LP + 511) // 512          # 26 final tiles


def _balance(a, b, cap):
    """Assign L dsts to NW bins of <= WIN slots, both edge sums <= cap.
    Returns pos[d] = bin*WIN + slot, or None if infeasible."""
    order = np.argsort(-(a + b), kind="stable")
    slots = np.zeros(NW, np.int64)
    suma = np.zeros(NW, np.int64)
    sumb = np.zeros(NW, np.int64)
    pos = np.empty(L, np.int64)
    big = 1 << 60
    for d in order:
        na = suma + a[d]
        nb = sumb + b[d]
        score = np.maximum(na, nb)
        score[(slots >= WIN) | (na > cap) | (nb > cap)] = big
        w = int(np.argmin(score))
        if score[w] >= big:
            return None
        pos[d] = w * WIN + slots[w]
        slots[w] += 1
        suma[w] += a[d]
        sumb[w] += b[d]
    return pos


def _rel_tables(row_t, col, vals, poss, B):
    """Bucket edges by (dst core, window) into [8, 128, NBLK] slot tables.
    vals: list of (per-edge array, pad value, np dtype)."""
    NBLK = NW * B
    shard = col // L
    d = (col % L).astype(np.int64)
    pos = np.empty(E, np.int64)
    for k in range(NCORES):
        m = shard == k
        pos[m] = poss[k][d[m]]
    w = pos // WIN
    s = pos % WIN
    key = shard * NW + w
    order = np.argsort(key, kind="stable")
    ks = key[order]
    counts = np.bincount(ks, minlength=NCORES * NW)
    starts = np.zeros(NCORES * NW, np.int64)
    np.cumsum(counts[:-1], out=starts[1:])
    pidx = np.arange(E) - starts[ks]
    blk = pidx // 128
    p = pidx % 128
    assert blk.max() < B, f"window overflow: {blk.max()} >= {B}"
    j = (ks % NW) * B + blk
    core = ks // NW
    gidx = np.full((NCORES, 128, NBLK), ZROW, np.int32)
    colw = np.full((NCORES, 128, NBLK), -1.0, NPBF16)
    gidx[core, p, j] = row_t[order]
    colw[core, p, j] = s[order].astype(NPBF16)
    out = [gidx, colw]
    for arr_e, fill, dt in vals:
        t = np.full((NCORES, 128, NBLK), fill, dt)
        t[core, p, j] = arr_e[order].astype(dt)
        out.append(t)
    return out


def host_prep(inputs):
    no_Nidx = np.asarray(inputs["no_Nidx"]).astype(np.int64)
    u_feature = np.asarray(inputs["u_feature"], dtype=np.float32)
    comment_x = np.ascontiguousarray(np.asarray(inputs["comment_x"],
                                                dtype=np.float32))
    edge_uu = np.asarray(inputs["edge_uu"]).astype(np.int64)
    cu_src = np.asarray(inputs["edge_cu_src"]).astype(np.int64)
    cu_dst = np.asarray(inputs["edge_cu_dst"]).astype(np.int64)
    emb_table = np.asarray(inputs["emb_table"], dtype=np.float32)

    uu_row, uu_col = edge_uu[0], edge_uu[1]
    deg_uu = np.bincount(uu_col, minlength=U).astype(np.int64)
    cnt_cu = np.maximum(np.bincount(cu_dst, minlength=U), 1).astype(np.int64)
    trow = np.concatenate([np.arange(U1, dtype=np.int64), EMB0 + no_Nidx])

    # per-core window balancing (B=5, fallback B=6)
    B = 5
    while True:
        poss = []
        ok = True
        for k in range(NCORES):
            lo, hi = k * L, (k + 1) * L
            a = np.bincount(uu_col[(uu_col >= lo) & (uu_col < hi)] - lo,
                            minlength=L)
            b = np.bincount(cu_dst[(cu_dst >= lo) & (cu_dst < hi)] - lo,
                            minlength=L)
            pos = _balance(a, b, B * 128)
            if pos is None:
                ok = False
                break
            poss.append(pos)
        if ok:
            break
        assert B == 5, "bin packing failed even at B=6"
        B = 6

    degp_e = (deg_uu[uu_row] * deg_uu[uu_col]).astype(np.float32)
    uu_gidx, uu_colw, uu_degp = _rel_tables(
        trow[uu_row], uu_col, [(degp_e, 0.0, np.float32)], poss, B)
    cntv_e = cnt_cu[cu_dst].astype(np.float32)
    cu_gidx, cu_colw, cu_cntv = _rel_tables(
        cu_src, cu_dst, [(cntv_e, 1.0, np.float32)], poss, B)

    # local row gather indices (sigma-ordered), padded to NLOCP*128
    loc_idx = np.full((NCORES, 128, NLOCP), ZROW, np.int32)
    for k in range(NCORES):
        inv = np.full(LP, -1, np.int64)
        inv[poss[k]] = np.arange(L)
        locr = np.full(NLOCP * 128, ZROW, np.int64)
        m = inv >= 0
        locr[:LP][m] = trow[k * L + inv[m]]
        loc_idx[k] = locr.reshape(NLOCP, 128).T

    # packed u_feature with ones column: [128, NFT*11]
    ufp = np.zeros((NFROWS, 11), np.float32)
    ufp[:U1, :10] = u_feature
    ufp[:U1, 10] = 1.0
    ufpack = np.ascontiguousarray(
        ufp.reshape(NFT, 128, 11).transpose(1, 0, 2).reshape(128, NFT * 11))

    embp = np.zeros((NFROWS, ED), np.float32)
    embp[:U2] = emb_table

    bsel = np.zeros((5, 2, 11), np.float32)
    for i, (src, _lo) in enumerate(EMB_BLOCKS):
        bsel[i, 0, src] = -1.0
        bsel[i, 1, src] = 1.0
        bsel[i, 0, 10] = 1.0
    kmat = np.zeros((11, ED), np.float32)
    for src, oc in PASS_COLS:
        kmat[src, oc] = 1.0
    iota62 = np.tile(np.arange(WIN, dtype=np.float32), (128, 1)).astype(NPBF16)
    ident = np.eye(128, dtype=np.float32).astype(NPBF16)

    shared = {
        "ufpack": ufpack,
        "embp": embp,
        "comment_x": comment_x,
        "e_tabs": np.stack([np.asarray(inputs[n], dtype=np.float32)
                            for n in ("e0", "e3", "e7", "e8", "e9")]),
        "gcn_w": np.asarray(inputs["gcn_w"], dtype=np.float32),
        "gcn_b": np.asarray(inputs["gcn_b"], dtype=np.float32),
        "sage_l_w": np.asarray(inputs["sage_l_w"], dtype=np.float32),
        "sage_l_b": np.asarray(inputs["sage_l_b"], dtype=np.float32),
        "sage_r_w": np.asarray(inputs["sage_r_w"], dtype=np.float32),
        "bn_gamma": np.asarray(inputs["bn_gamma"], dtype=np.float32),
        "bn_beta": np.asarray(inputs["bn_beta"], dtype=np.float32),
        "bsel": bsel,
        "kmat": kmat,
        "iota62": iota62,
        "ident": ident,
    }
    percore = []
    for k in range(NCORES):
        m = dict(shared)
        m["uu_gidx"] = uu_gidx[k]
        m["uu_colw"] = uu_colw[k]
        m["uu_degp"] = uu_degp[k]
        m["cu_gidx"] = cu_gidx[k]
        m["cu_colw"] = cu_colw[k]
        m["cu_cntv"] = cu_cntv[k]
        m["loc_idx"] = loc_idx[k]
        percore.append(m)
    cfg = {"B": B, "NBLK": NW * B, "poss": poss}
    return percore, cfg


INPUT_SPECS = [
    ("ufpack", (128, NFT * 11), F32),
    ("embp", (NFROWS, ED), F32),
    ("comment_x", (C, DC), F32),
    ("e_tabs", (5, 2, 16), F32),
    ("gcn_w", (NH, ED, H), F32),
    ("gcn_b", (NH, H), F32),
    ("sage_l_w", (NH, DC, H), F32),
    ("sage_l_b", (NH, H), F32),
    ("sage_r_w", (NH, ED, H), F32),
    ("bn_gamma", (H,), F32),
    ("bn_beta", (H,), F32),
    ("bsel", (5, 2, 11), F32),
    ("kmat", (11, ED), F32),
    ("iota62", (128, WIN), BF16),
    ("ident", (128, 128), BF16),
    ("loc_idx", (128, NLOCP), I32),
]


def build(nc, tc, io, out_ap, cfg):
    B = cfg["B"]
    NBLK = NW * B
    GK = 4 * B            # blocks per gather call / S group (4 windows)
    NGRP = NW // 4        # 52 groups per relation
    GW = 4 * WIN          # 248 agg columns per group
    AT = mybir.AluOpType
    AF = mybir.ActivationFunctionType
    AX = mybir.AxisListType
    RG = [list(range(NCORES))]

    T = nc.dram_tensor("T_d", [TROWS, ED], BF16).ap()
    bn_in = nc.dram_tensor("bn_in_d", [H, 2], F32).ap()
    bn_out = nc.dram_tensor("bn_out_d", [H, 2], F32, addr_space="Shared").ap()

    import contextlib
    stack = contextlib.ExitStack()
    big = stack.enter_context(tc.tile_pool(name="big", bufs=1))
    agg_u = big.tile([ED, LP], BF16, tag="agg_u")
    agg_c = big.tile([DC, LP], BF16, tag="agg_c")
    iota_sb = big.tile([128, WIN], BF16, tag="iota_sb")
    ident_sb = big.tile([128, 128], BF16, tag="ident_sb")
    mp16 = big.tile([11, ED], BF16, tag="mp16")
    wg_sb = [big.tile([ED, H], BF16, name=f"wg{h}", tag=f"wg{h}") for h in range(NH)]
    wr_sb = [big.tile([ED, H], BF16, name=f"wr{h}", tag=f"wr{h}") for h in range(NH)]
    wl_sb = [big.tile([DC, H], BF16, name=f"wl{h}", tag=f"wl{h}") for h in range(NH)]
    bh_sb = [big.tile([H, 1], F32, name=f"bh{h}", tag=f"bh{h}") for h in range(NH)]
    nbh_sb = [big.tile([H, 1], F32, name=f"nbh{h}", tag=f"nbh{h}") for h in range(NH)]
    gam_sb = big.tile([H, 1], F32, tag="gam")
    bet_sb = big.tile([H, 1], F32, tag="bet")
    uu_gidx_sb = big.tile([128, NBLK], I32, tag="uu_gidx")
    uu_colw_sb = big.tile([128, NBLK], BF16, tag="uu_colw")
    cu_gidx_sb = big.tile([128, NBLK], I32, tag="cu_gidx")
    cu_colw_sb = big.tile([128, NBLK], BF16, tag="cu_colw")
    edis_sb = big.tile([128, NBLK], BF16, tag="edis")
    cinv_sb = big.tile([128, NBLK], BF16, tag="cinv")
    loc_idx_sb = big.tile([128, NLOCP], I32, tag="loc_idx")
    uu_gidx2 = big.tile([128, NBLK], I32, tag="uu_gidx2")
    loc_idx2 = big.tile([128, NLOCP], I32, tag="loc_idx2")
    s_part = big.tile([H, NT], F32, tag="s_part")
    sq_part = big.tile([H, NT], F32, tag="sq_part")

    nc.sync.dma_start(out=iota_sb[:], in_=io["iota62"])
    nc.sync.dma_start(out=ident_sb[:], in_=io["ident"])
    nc.sync.dma_start(out=uu_gidx_sb[:], in_=io["uu_gidx"])
    nc.sync.dma_start(out=uu_colw_sb[:], in_=io["uu_colw"])
    nc.sync.dma_start(out=cu_gidx_sb[:], in_=io["cu_gidx"])
    nc.sync.dma_start(out=cu_colw_sb[:], in_=io["cu_colw"])
    nc.sync.dma_start(out=loc_idx_sb[:], in_=io["loc_idx"])
    nc.sync.dma_start(out=gam_sb[:], in_=io["bn_gamma"][:, None])
    nc.sync.dma_start(out=bet_sb[:], in_=io["bn_beta"][:, None])

    # ---- small prep: weights->bf16, M', biases, edis, 1/cnt --------------
    with (
        tc.tile_pool(name="prep", bufs=2) as prep,
        tc.tile_pool(name="prepp", bufs=2, space="PSUM") as prepp,
    ):
        for h in range(NH):
            for name, dst, rows in (("gcn_w", wg_sb[h], ED),
                                    ("sage_r_w", wr_sb[h], ED),
                                    ("sage_l_w", wl_sb[h], DC)):
                t = prep.tile([ED, H], F32, tag=f"wld_{name}")
                nc.sync.dma_start(out=t[:rows, :], in_=io[name][h])
                nc.vector.tensor_copy(out=dst[:], in_=t[:rows, :])
            t1 = prep.tile([H, 1], F32, tag="t1")
            t2 = prep.tile([H, 1], F32, tag="t2")
            nc.sync.dma_start(out=t1[:], in_=io["gcn_b"][h][:, None])
            nc.sync.dma_start(out=t2[:], in_=io["sage_l_b"][h][:, None])
            nc.vector.tensor_tensor(out=bh_sb[h][:], in0=t1[:], in1=t2[:],
                                    op=AT.add)
            nc.vector.tensor_scalar(out=nbh_sb[h][:], in0=bh_sb[h][:],
                                    scalar1=-1.0, scalar2=None, op0=AT.mult)
        mpf = prep.tile([11, ED], F32, tag="mpf")
        nc.sync.dma_start(out=mpf[:], in_=io["kmat"])
        for i, (_src, lo) in enumerate(EMB_BLOCKS):
            e_sb = prep.tile([2, 16], F32, tag="e_sb")
            b_sb = prep.tile([2, 11], F32, tag="b_sb")
            nc.sync.dma_start(out=e_sb[:], in_=io["e_tabs"][i])
            nc.sync.dma_start(out=b_sb[:], in_=io["bsel"][i])
            mpp = prepp.tile([11, 16], F32, tag="mpp")
            nc.tensor.matmul(out=mpp[:], lhsT=b_sb[:], rhs=e_sb[:],
                             start=True, stop=True)
            nc.vector.tensor_copy(out=mpf[:, lo:lo + 16], in_=mpp[:])
        nc.vector.tensor_copy(out=mp16[:], in_=mpf[:])
        # edis = (degp>0) * rsqrt(max(degp,1)), degp = deg[row]*deg[col]
        degp = prep.tile([128, NBLK], F32, tag="degp")
        nc.sync.dma_start(out=degp[:], in_=io["uu_degp"])
        dmx = prep.tile([128, NBLK], F32, tag="dmx")
        nc.vector.tensor_scalar(out=dmx[:], in0=degp[:], scalar1=1.0,
                                scalar2=None, op0=AT.max)
        drc = prep.tile([128, NBLK], F32, tag="drc")
        nc.vector.reciprocal(out=drc[:], in_=dmx[:])
        dsq = prep.tile([128, NBLK], F32, tag="dsq")
        nc.scalar.activation(out=dsq[:], in_=drc[:], func=AF.Sqrt)
        dmk = prep.tile([128, NBLK], F32, tag="dmk")
        nc.vector.tensor_scalar(out=dmk[:], in0=degp[:], scalar1=0.0,
                                scalar2=None, op0=AT.is_gt)
        nc.vector.tensor_tensor(out=edis_sb[:], in0=dsq[:], in1=dmk[:],
                                op=AT.mult)
        cntv = prep.tile([128, NBLK], F32, tag="cntv")
        nc.sync.dma_start(out=cntv[:], in_=io["cu_cntv"])
        cinvf = prep.tile([128, NBLK], F32, tag="cinvf")
        nc.vector.reciprocal(out=cinvf[:], in_=cntv[:])
        nc.vector.tensor_copy(out=cinv_sb[:], in_=cinvf[:])

    # ---- scatter phases: one-hot matmul aggregation ----------------------
    def scatter_phase(src_ap, src_cols, gidx_sb, colw_sb, wgt_sb, agg, prefix):
        with (
            tc.tile_pool(name=f"{prefix}g", bufs=8) as gp,
            tc.tile_pool(name=f"{prefix}s", bufs=4) as sp,
            tc.tile_pool(name=f"{prefix}p", bufs=3, space="PSUM") as pp_pool,
        ):
            for g in range(NGRP):
                j0 = g * GK
                seq = sp.tile([128, GK * WIN], BF16, tag=f"{prefix}seq")
                nc.vector.tensor_tensor(
                    out=seq[:].rearrange("p (k c) -> p k c", c=WIN),
                    in0=iota_sb[:].unsqueeze(1).broadcast_to([128, GK, WIN]),
                    in1=colw_sb[:, j0:j0 + GK].unsqueeze(2).broadcast_to(
                        [128, GK, WIN]),
                    op=AT.is_equal)
                # weight by per-edge scalar (edis or 1/cnt)
                sw = sp.tile([128, GK * WIN], BF16, tag=f"{prefix}sw")
                nc.vector.tensor_tensor(
                    out=sw[:].rearrange("p (k c) -> p k c", c=WIN),
                    in0=seq[:].rearrange("p (k c) -> p k c", c=WIN),
                    in1=wgt_sb[:, j0:j0 + GK].unsqueeze(2).broadcast_to(
                        [128, GK, WIN]),
                    op=AT.mult)
                pp = pp_pool.tile([src_cols, GW], F32, tag=f"{prefix}pp")
                st = gp.tile([128, GK * src_cols], BF16, tag=f"{prefix}st")
                for wl in range(4):
                    for b in range(B):
                        kk = wl * B + b
                        nc.gpsimd.indirect_dma_start(
                            out=st[:, kk * src_cols:(kk + 1) * src_cols],
                            out_offset=None, in_=src_ap,
                            in_offset=IndirectOffsetOnAxis(
                                ap=gidx_sb[:, j0 + kk:j0 + kk + 1], axis=0))
                        nc.tensor.matmul(
                            out=pp[:, wl * WIN:(wl + 1) * WIN],
                            lhsT=st[:, kk * src_cols:(kk + 1) * src_cols],
                            rhs=sw[:, kk * WIN:(kk + 1) * WIN],
                            start=(b == 0), stop=(b == B - 1))
                nc.scalar.activation(out=agg[:, g * GW:(g + 1) * GW],
                                     in_=pp[:], func=AF.Copy)

    scatter_phase(io["comment_x"], DC, cu_gidx_sb, cu_colw_sb, cinv_sb,
                  agg_c, "cu")

    # ---- build T: newF rows + emb cast + zero row ------------------------
    with (
        tc.tile_pool(name="ufb", bufs=1) as ufb,
        tc.tile_pool(name="ufc", bufs=3) as ufc,
        tc.tile_pool(name="ufbp", bufs=2, space="PSUM") as ufbp,
    ):
        ufpk = ufb.tile([128, NFT * 11], F32, tag="ufpk")
        nc.sync.dma_start(out=ufpk[:], in_=io["ufpack"])
        ufb16 = ufb.tile([128, NFT * 11], BF16, tag="ufb16")
        nc.vector.tensor_copy(out=ufb16[:], in_=ufpk[:])
        ngrp4 = (NFT + 3) // 4
        for g4 in range(ngrp4):
            n0 = 4 * g4
            nt4 = min(4, NFT - n0)
            pn = ufbp.tile([128, 4 * ED], F32, tag="pn4")
            for i in range(nt4):
                tp = ufbp.tile([11, 128], BF16, tag="tp1", bufs=4)
                nc.tensor.transpose(out=tp[:],
                                    in_=ufb16[:, 11 * (n0 + i):11 * (n0 + i + 1)],
                                    identity=ident_sb[:])
                tpsb = ufc.tile([11, 128], BF16, tag="tpsb", bufs=4)
                if i % 2 == 0:
                    nc.vector.tensor_copy(out=tpsb[:], in_=tp[:])
                else:
                    nc.scalar.activation(out=tpsb[:], in_=tp[:], func=AF.Copy)
                nc.tensor.matmul(out=pn[:, ED * i:ED * (i + 1)],
                                 lhsT=tpsb[:],
                                 rhs=mp16[:], start=True, stop=True)
            nf = ufc.tile([128, 4 * ED], BF16, tag="nf4")
            if g4 % 2 == 0:
                nc.scalar.activation(out=nf[:, :ED * nt4], in_=pn[:, :ED * nt4],
                                     func=AF.Copy)
            else:
                nc.vector.tensor_copy(out=nf[:, :ED * nt4], in_=pn[:, :ED * nt4])
            nc.sync.dma_start(
                out=T[128 * n0:128 * (n0 + nt4), :].rearrange(
                    "(n p) c -> p n c", p=128),
                in_=nf[:, :ED * nt4].rearrange("p (n c) -> p n c", c=ED))
        # emb cast: flat f32 -> bf16 (row-major preserved)
        embf = io["embp"].rearrange("r c -> (r c)")
        tflat = T.rearrange("r c -> (r c)")
        toff = EMB0 * ED
        per = NFROWS * ED // 128          # 33235 elems per partition total
        widths = [8309, 8309, 8309, 8308]
        off = 0
        for ci, wd in enumerate(widths):
            src = ufc.tile([128, 8309], F32, tag="embsrc")
            nc.sync.dma_start(
                out=src[:, :wd],
                in_=embf[off * 128:(off + wd) * 128].rearrange(
                    "(p x) -> p x", p=128))
            cst = ufc.tile([128, 8309], BF16, tag="embcst")
            if ci % 2 == 0:
                nc.vector.tensor_copy(out=cst[:, :wd], in_=src[:, :wd])
            else:
                nc.scalar.activation(out=cst[:, :wd], in_=src[:, :wd],
                                     func=AF.Copy)
            nc.sync.dma_start(
                out=tflat[toff + off * 128:toff + (off + wd) * 128].rearrange(
                    "(p x) -> p x", p=128),
                in_=cst[:, :wd])
            off += wd
        assert off == per
        zr = ufc.tile([8, ED], BF16, tag="zrow")
        nc.vector.memset(zr[:], 0.0)
        nc.sync.dma_start(out=T[TR:TROWS, :], in_=zr[:])


    # fence: chain gather indices through a readback of T's zero rows so the
    # T-reading gathers cannot be scheduled before the T writes complete
    tflat2 = T.rearrange("r c -> (r c)")
    with tc.tile_pool(name="fence", bufs=1) as fp:
        tokf = fp.tile([128, 5], BF16, tag="tokf")
        nc.sync.dma_start(
            out=tokf[:],
            in_=tflat2[TR * ED:TR * ED + 640].rearrange("(p x) -> p x", p=128))
        toki = fp.tile([128, 5], I32, tag="toki")
        nc.vector.tensor_copy(out=toki[:], in_=tokf[:])
        nc.vector.tensor_tensor(out=uu_gidx2[:], in0=uu_gidx_sb[:],
                                in1=toki[:, 0:1].broadcast_to([128, NBLK]),
                                op=AT.add)
        nc.vector.tensor_tensor(out=loc_idx2[:], in0=loc_idx_sb[:],
                                in1=toki[:, 0:1].broadcast_to([128, NLOCP]),
                                op=AT.add)

    scatter_phase(T, ED, uu_gidx2, uu_colw_sb, edis_sb, agg_u, "uu")

    # ---- final: local gather, matmuls, leaky relu, BN stats --------------
    nodep_cm = tc.tile_pool(name="nodep", bufs=1)
    nodep = nodep_cm.__enter__()
    node = nodep.tile([H, NT * 512], F32, tag="node")
    ufg_all = nodep.tile([128, NLOCP * ED], BF16, tag="ufg_all")
    with (
        tc.tile_pool(name="fin", bufs=2) as fin,
        tc.tile_pool(name="finp", bufs=2, space="PSUM") as finp,
    ):
        for ci in range(NLOCG):
            nc.gpsimd.indirect_dma_start(
                out=ufg_all[:, ci * ED:(ci + 1) * ED],
                out_offset=None, in_=T,
                in_offset=IndirectOffsetOnAxis(
                    ap=loc_idx2[:, ci:ci + 1], axis=0))
        for t in range(NT):
            c0 = 512 * t
            wt = min(512, LP - c0)
            ns = (wt + 127) // 128
            tpp = finp.tile([ED, 512], BF16, tag="tpp")
            for s in range(ns):
                su = 4 * t + s
                nc.tensor.transpose(out=tpp[:, 128 * s:128 * (s + 1)],
                                    in_=ufg_all[:, su * ED:(su + 1) * ED],
                                    identity=ident_sb[:])
            ufT = fin.tile([ED, 512], BF16, tag="ufT")
            nc.scalar.activation(out=ufT[:, :128 * ns], in_=tpp[:, :128 * ns],
                                 func=AF.Copy)
            rel = []
            for h in range(NH):
                ph = finp.tile([H, 512], F32, tag="ph")
                nc.tensor.matmul(out=ph[:, :wt], lhsT=wg_sb[h][:],
                                 rhs=agg_u[:, c0:c0 + wt], start=True,
                                 stop=False)
                nc.tensor.matmul(out=ph[:, :wt], lhsT=wr_sb[h][:],
                                 rhs=ufT[:, :wt], start=False, stop=False)
                nc.tensor.matmul(out=ph[:, :wt], lhsT=wl_sb[h][:],
                                 rhs=agg_c[:, c0:c0 + wt], start=False,
                                 stop=True)
                rp = fin.tile([H, 512], F32, tag="rp")
                nc.scalar.activation(out=rp[:, :wt], in_=ph[:, :wt],
                                     func=AF.Relu, bias=bh_sb[h][:])
                rn = fin.tile([H, 512], F32, tag="rn")
                nc.scalar.activation(out=rn[:, :wt], in_=ph[:, :wt],
                                     func=AF.Relu, bias=nbh_sb[h][:],
                                     scale=-1.0)
                rel.append((rp, rn))
            a1 = fin.tile([H, 512], F32, tag="a1")
            nc.vector.tensor_tensor(out=a1[:, :wt], in0=rel[0][0][:, :wt],
                                    in1=rel[1][0][:, :wt], op=AT.add)
            a2 = fin.tile([H, 512], F32, tag="a2")
            nc.vector.tensor_tensor(out=a2[:, :wt], in0=rel[0][1][:, :wt],
                                    in1=rel[1][1][:, :wt], op=AT.add)
            a3 = fin.tile([H, 512], F32, tag="a3")
            nc.vector.tensor_scalar(out=a3[:, :wt], in0=a2[:, :wt],
                                    scalar1=-0.3, scalar2=None, op0=AT.mult)
            nc.vector.tensor_tensor(out=node[:, c0:c0 + wt], in0=a1[:, :wt],
                                    in1=a3[:, :wt], op=AT.add)
            nc.vector.tensor_reduce(out=s_part[:, t:t + 1],
                                    in_=node[:, c0:c0 + wt], axis=AX.X,
                                    op=AT.add)
            sqs = fin.tile([H, 512], F32, tag="sqs")
            nc.scalar.activation(out=sqs[:, :wt], in_=node[:, c0:c0 + wt],
                                 func=AF.Square,
                                 accum_out=sq_part[:, t:t + 1])

    # ---- BN: stats correction, allreduce, normalize, write ---------------
    with tc.tile_pool(name="bn", bufs=2) as bn:
        # v_empty = sum_h leaky(bh[h]): value of unassigned slots
        vp = bn.tile([H, 2], F32, tag="vp")
        vn = bn.tile([H, 2], F32, tag="vn")
        for h in range(NH):
            nc.scalar.activation(out=vp[:, h:h + 1], in_=bh_sb[h][:],
                                 func=AF.Relu)
            nc.scalar.activation(out=vn[:, h:h + 1], in_=bh_sb[h][:],
                                 func=AF.Relu, scale=-1.0)
        vps = bn.tile([H, 1], F32, tag="vps")
        nc.vector.tensor_tensor(out=vps[:], in0=vp[:, 0:1], in1=vp[:, 1:2],
                                op=AT.add)
        vns = bn.tile([H, 1], F32, tag="vns")
        nc.vector.tensor_tensor(out=vns[:], in0=vn[:, 0:1], in1=vn[:, 1:2],
                                op=AT.add)
        vn3 = bn.tile([H, 1], F32, tag="vn3")
        nc.vector.tensor_scalar(out=vn3[:], in0=vns[:], scalar1=-0.3,
                                scalar2=None, op0=AT.mult)
        ve = bn.tile([H, 1], F32, tag="ve")
        nc.vector.tensor_tensor(out=ve[:], in0=vps[:], in1=vn3[:], op=AT.add)
        ve2 = bn.tile([H, 1], F32, tag="ve2")
        nc.vector.tensor_tensor(out=ve2[:], in0=ve[:], in1=ve[:], op=AT.mult)

        raw = bn.tile([H, 2], F32, tag="rawstat")
        nc.vector.tensor_reduce(out=raw[:, 0:1], in_=s_part[:], axis=AX.X,
                                op=AT.add)
        nc.vector.tensor_reduce(out=raw[:, 1:2], in_=sq_part[:], axis=AX.X,
                                op=AT.add)
        corr = bn.tile([H, 2], F32, tag="corr")
        nc.vector.tensor_scalar(out=corr[:, 0:1], in0=ve[:],
                                scalar1=-float(NEMPTY), scalar2=None,
                                op0=AT.mult)
        nc.vector.tensor_scalar(out=corr[:, 1:2], in0=ve2[:],
                                scalar1=-float(NEMPTY), scalar2=None,
                                op0=AT.mult)
        stat = bn.tile([H, 2], F32, tag="stat")
        nc.vector.tensor_tensor(out=stat[:], in0=raw[:], in1=corr[:],
                                op=AT.add)
        nc.sync.dma_start(out=bn_in, in_=stat[:])
        nc.gpsimd.collective_compute(
            "AllReduce", mybir.AluOpType.add, replica_groups=RG,
            ins=[bn_in], outs=[bn_out])
        gstat = bn.tile([H, 2], F32, tag="gstat")
        nc.sync.dma_start(out=gstat[:], in_=bn_out)
        mean = bn.tile([H, 1], F32, tag="mean")
        nc.vector.tensor_scalar(out=mean[:], in0=gstat[:, 0:1],
                                scalar1=1.0 / U, scalar2=None, op0=AT.mult)
        ex2 = bn.tile([H, 1], F32, tag="ex2")
        nc.vector.tensor_scalar(out=ex2[:], in0=gstat[:, 1:2],
                                scalar1=1.0 / U, scalar2=None, op0=AT.mult)
        m2 = bn.tile([H, 1], F32, tag="m2")
        nc.vector.tensor_tensor(out=m2[:], in0=mean[:], in1=mean[:], op=AT.mult)
        var = bn.tile([H, 1], F32, tag="var")
        nc.vector.tensor_tensor(out=var[:], in0=ex2[:], in1=m2[:],
                                op=AT.subtract)
        vd = bn.tile([H, 1], F32, tag="vd")
        nc.vector.tensor_scalar(out=vd[:], in0=var[:], scalar1=1e-5,
                                scalar2=None, op0=AT.add)
        rv = bn.tile([H, 1], F32, tag="rv")
        nc.vector.reciprocal(out=rv[:], in_=vd[:])
        rs = bn.tile([H, 1], F32, tag="rs")
        nc.scalar.activation(out=rs[:], in_=rv[:], func=AF.Sqrt)
        asc = bn.tile([H, 1], F32, tag="asc")
        nc.vector.tensor_tensor(out=asc[:], in0=rs[:], in1=gam_sb[:],
                                op=AT.mult)
        mb = bn.tile([H, 1], F32, tag="mb")
        nc.vector.tensor_tensor(out=mb[:], in0=mean[:], in1=asc[:], op=AT.mult)
        bsh = bn.tile([H, 1], F32, tag="bsh")
        nc.vector.tensor_tensor(out=bsh[:], in0=bet_sb[:], in1=mb[:],
                                op=AT.subtract)
        for t in range(NT):
            c0 = 512 * t
            wt = min(512, LP - c0)
            yt = bn.tile([H, 512], F32, tag="yt")
            nc.vector.tensor_scalar(out=yt[:, :wt], in0=node[:, c0:c0 + wt],
                                    scalar1=asc[:], scalar2=bsh[:],
                                    op0=AT.mult, op1=AT.add)
            nc.sync.dma_start(out=out_ap[:, c0:c0 + wt], in_=yt[:, :wt])

    nodep_cm.__exit__(None, None, None)
    stack.close()


def make_nc(cfg):
    nc = bacc.Bacc("TRN2", target_bir_lowering=False, debug=False,
                   enable_asserts=False, num_devices=NCORES)
    io = {}
    for name, shape, dt in INPUT_SPECS:
        io[name] = nc.dram_tensor(name, list(shape), dt,
                                  kind="ExternalInput").ap()
    NBLK = cfg["NBLK"]
    for name, dt in (("uu_gidx", I32), ("uu_colw", BF16), ("uu_degp", F32),
                     ("cu_gidx", I32), ("cu_colw", BF16), ("cu_cntv", F32)):
        io[name] = nc.dram_tensor(name, [128, NBLK], dt,
                                  kind="ExternalInput").ap()
    out_ap = nc.dram_tensor("out_shard", [H, LP], F32,
                            kind="ExternalOutput").ap()
    with tile.TileContext(nc) as tc:
        build(nc, tc, io, out_ap, cfg)
    nc.compile()
    return nc


def kernel(**inputs):
    percore, cfg = host_prep(inputs)
    nc = make_nc(cfg)
    res = bass_utils.run_bass_kernel_spmd(nc, percore,
                                          core_ids=list(range(NCORES)))
    out = np.empty((U, H), np.float32)
    for k in range(NCORES):
        arr = np.asarray(res.results[k]["out_shard"])  # [H, LP]
        out[k * L:(k + 1) * L] = arr.T[cfg["poss"][k]]
    return out
